# revision 18
# baseline (speedup 1.0000x reference)
"""Trainium2 Bass kernel: batched 3x3 polar decomposition + tangent projection.

reference semantics (per matrix n of N=2,000,000):
    u, _, vT = svd(x);  xm = u @ vT          (polar factor)
    vt = 0.5*(v - xm @ v^T @ xm)

Implementation: determinant-scaled Newton iteration for the polar factor
(gamma-form, scale-invariant):  X <- X + sign(d)|d|^(-1/3) * cof(X)
with cof() the signed cofactor matrix (X^{-T} = cof(X)/det(X)); final
iteration applies exact alpha*X + beta*cof(X) with an extra 1/sqrt(2)
folded in so the projection needs no 0.5 on the quadratic term:
    vt = 0.5 v - xmh (xmh^T v)^T,   xmh = xm/sqrt(2).

Data layout: SoA "planes" [128, 3, 3, F] per tile; the cyclic cofactor
index patterns are expressed with negative-stride access patterns
(rows (2,0) = start 2, step -2), split into 2x2 blocks per product.

Each tile's columns are split between the Vector engine (DVE) and GPSIMD,
which run the whole pipeline independently on their column ranges (fp32
tensor_tensor on DVE never takes the shared SBUF port, so both engines
stream concurrently); the Ln/Exp scalar chains run on the Scalar engine.

Sharding: batch split evenly across 8 NeuronCores, zero communication.
"""

import numpy as np

import concourse.bass as bass
import concourse.bacc as bacc
import concourse.mybir as mybir
import concourse.tile as tile
from concourse.bass_utils import run_bass_kernel_spmd

dt = mybir.dt.float32
AF = mybir.ActivationFunctionType
OP = mybir.AluOpType

NCORES = 8
N_TOTAL = 2_000_000
N_CORE = N_TOTAL // NCORES      # 250_000

# device tiling (full config)
F = 652                          # free-dim elements per partition per tile
TILES = 3
ITERS = 5                        # total Newton iterations (incl. final)
FG = 211                         # columns of each tile handled by GPSIMD

LN2 = float(np.log(2.0))
DELTA = 1e-15                    # det bump (unsticks exact-zero fp32 det)
EPS = 1e-35                      # clamp inside Ln


def _pipeline(nc, eng, lo, hi, X4, vb4, C, Tb, Wf, sc, c_eps, c_b2, c_dl, iters, Cps=None):
    """Emit the full per-tile computation for columns [lo:hi) on engine
    `eng` (nc.vector or nc.gpsimd). `sc` maps name -> [128, f] scalar tile.

    When `Cps` (a [128,3,3,hi-lo] PSUM tile) is given (DVE pipeline), the
    cofactor lives in PSUM *negated* (Cps = Tb - Ta = -cof); since gamma and
    beta are odd in det and det is computed from Cps, the two sign flips
    cancel identically. One operand of most DVE ops then comes through the
    dedicated PSUM port, leaving the shared SBUF port to GPSIMD.
    """
    fp = hi - lo
    s = lambda name: sc[name][:, lo:hi]
    X = X4[:, :, :, lo:hi]
    vb = vb4[:, :, :, lo:hi]
    Cp = Cps if Cps is not None else C[:, :, :, lo:hi]
    Tp = Tb[:, :, :, lo:hi]
    Wp = Wf[:, :, :, lo:hi]
    shp = (128, 3, 3, fp)
    psum = Cps is not None

    r12 = lambda a: a[:, 1:3, :, :]
    r20 = lambda a: a[:, 2::-2, :, :]
    r0 = lambda a: a[:, 0:1, :, :]
    r1 = lambda a: a[:, 1:2, :, :]
    c12 = lambda a: a[:, :, 1:3, :]
    c20 = lambda a: a[:, :, 2::-2, :]
    c0 = lambda a: a[:, :, 0:1, :]
    c1 = lambda a: a[:, :, 1:2, :]

    for it in range(iters):
        last = it == iters - 1

        # signed cofactor: cof = X[r1,c1]X[r2,c2] - X[r1,c2]X[r2,c1]
        # (psum path stores Cp := Tp - Ta = -cof)
        eng.tensor_mul(Cp[:, 0:2, 0:2, :], c12(r12(X)), c20(r20(X)))
        eng.tensor_mul(Cp[:, 0:2, 2:3, :], c0(r12(X)), c1(r20(X)))
        eng.tensor_mul(Cp[:, 2:3, 0:2, :], c12(r0(X)), c20(r1(X)))
        eng.tensor_mul(Cp[:, 2:3, 2:3, :], c0(r0(X)), c1(r1(X)))
        eng.tensor_mul(Tp[:, 0:2, 0:2, :], c20(r12(X)), c12(r20(X)))
        eng.tensor_mul(Tp[:, 0:2, 2:3, :], c1(r12(X)), c0(r20(X)))
        eng.tensor_mul(Tp[:, 2:3, 0:2, :], c20(r0(X)), c12(r1(X)))
        eng.tensor_mul(Tp[:, 2:3, 2:3, :], c1(r0(X)), c0(r1(X)))
        if psum:
            eng.tensor_sub(Cp, Tp, Cp)          # Cp := -cof  (in1/out PSUM)
        else:
            eng.tensor_sub(Cp, Cp, Tp)          # Cp := +cof

        # det = sum_j X[0,j]*Cp[0,j] (+ DELTA bump); sign flip is harmless
        D = sc["D"][:, :, lo:hi]
        eng.tensor_mul(D, X[:, 0, :, :], Cp[:, 0, :, :])
        eng.tensor_add(s("tq"), D[:, 0, :], D[:, 1, :])
        if eng is nc.vector:
            eng.scalar_tensor_tensor(s("ds"), s("tq"), DELTA, D[:, 2, :], OP.add, OP.add)
            eng.tensor_mul(s("d2"), s("ds"), s("ds"))
        else:
            eng.tensor_add(s("tq"), s("tq"), D[:, 2, :])
            dlb = c_dl.broadcast_to((128, fp))
            eng.tensor_add(s("ds"), s("tq"), dlb)
            eng.tensor_mul(s("d2"), s("ds"), s("ds"))
        nc.scalar.activation(s("L"), s("d2"), AF.Ln, bias=c_eps[:, :])

        if not last:
            # gamma = ds * exp(-2/3 * L)
            nc.scalar.activation(s("w"), s("L"), AF.Exp, scale=-2.0 / 3.0)
            eng.tensor_mul(s("ga"), s("ds"), s("w"))
            gb = s("ga").unsqueeze(1).unsqueeze(1).broadcast_to(shp)
            if psum:
                eng.tensor_mul(Cp, gb, Cp)      # Cp := gamma * Cp (in place)
                eng.tensor_add(X, X, Cp)
            else:
                eng.tensor_mul(Tp, Cp, gb)
                eng.tensor_add(X, X, Tp)
        else:
            # xm = alpha*X + beta*Cp (full scale)
            nc.scalar.activation(s("al"), s("L"), AF.Exp, scale=-1.0 / 6.0, bias=c_b2[:, :])
            nc.scalar.activation(s("w"), s("L"), AF.Exp, scale=-5.0 / 6.0, bias=c_b2[:, :])
            eng.tensor_mul(s("be"), s("ds"), s("w"))
            ab = s("al").unsqueeze(1).unsqueeze(1).broadcast_to(shp)
            bb = s("be").unsqueeze(1).unsqueeze(1).broadcast_to(shp)
            if psum:
                eng.tensor_mul(Cp, bb, Cp)      # beta * Cp (in place)
                eng.tensor_mul(Tp, X, ab)
                eng.tensor_add(Cp, Tp, Cp)      # xm (in PSUM)
            else:
                eng.tensor_mul(Tp, X, ab)
                eng.tensor_mul(Cp, Cp, bb)
                eng.tensor_add(Cp, Tp, Cp)
            # Cp now holds xm

    # tangent projection: vt = vh - xm (xm^T vh)^T,  vh = v/2
    for k in range(3):
        # Wf[k,j] = sum_i xm[i,k]*vh[i,j]
        ck = Cp[:, 0:3, k : k + 1, :].broadcast_to(shp)
        if psum:
            eng.tensor_mul(Tp, vb, ck)
        else:
            eng.tensor_mul(Tp, ck, vb)
        eng.tensor_add(Wp[:, k, :, :], Tp[:, 0, :, :], Tp[:, 1, :, :])
        eng.tensor_add(Wp[:, k, :, :], Wp[:, k, :, :], Tp[:, 2, :, :])
    for k in range(3):
        # P[i,j] = xm[i,k]*Wf[j,k];  out = vh - sum_k P
        cki = Cp[:, 0:3, k : k + 1, :].broadcast_to(shp)
        wkb = Wp[:, 0:3, k, :].unsqueeze(1).broadcast_to(shp)
        if psum:
            eng.tensor_mul(Tp, wkb, cki)
        else:
            eng.tensor_mul(Tp, cki, wkb)
        eng.tensor_sub(vb, vb, Tp)


def _patch_act_tables():
    """Steer the ACT table-load pass so Ln and Exp resolve to the single
    combined set (natural_log_exp_and_others); otherwise the pass picks
    separate sets and every iteration thrashes ~2.7us table loads."""
    keep = "natural_log_exp_and_others"
    orig = bacc.get_activation_tables

    def patched(arch):
        tabs = orig(arch)
        return {
            name: (funcs if name == keep else funcs - {AF.Ln, AF.Exp})
            for name, funcs in tabs.items()
        }

    bacc.get_activation_tables = patched


_patch_act_tables()


def build_nc(f=F, tiles=TILES, iters=ITERS, fg=FG):
    """Per-core Bass graph. Inputs x, v: [9, tiles*128*f] f32 planes (plane
    p = 3*i+j holds entry (i,j) of each matrix, matrix m at column m);
    output "out" same layout holding vt."""
    npt = 128 * f
    np_tot = npt * tiles
    fd = f - fg                    # DVE columns [0:fd), GPSIMD [fd:f)

    nc = bacc.Bacc()
    x = nc.declare_dram_parameter("x", [9, np_tot], dt, isOutput=False)
    v = nc.declare_dram_parameter("v", [9, np_tot], dt, isOutput=False)
    out = nc.declare_dram_parameter("out", [9, np_tot], dt, isOutput=True)

    scalar_names = ["tq", "ds", "d2", "L", "w", "ga", "al", "be"]

    with tile.TileContext(nc) as tc:
        with tc.tile_pool(name="p", bufs=1) as pool, \
             tc.tile_pool(name="ps", bufs=1, space="PSUM") as psp:
            c_eps = pool.tile([128, 1], dt, tag="c_eps")
            c_b2 = pool.tile([128, 1], dt, tag="c_b2")
            c_dl = pool.tile([128, 1], dt, tag="c_dl")
            nc.vector.memset(c_eps[:, :], EPS)
            nc.vector.memset(c_b2[:, :], -LN2)
            nc.vector.memset(c_dl[:, :], DELTA)
            for t in range(tiles):
                sl = slice(t * npt, (t + 1) * npt)
                xsrc = x[:, sl].rearrange("p (q e) -> q p e", q=128)
                vsrc = v[:, sl].rearrange("p (q e) -> q p e", q=128)
                osrc = out[:, sl].rearrange("p (q e) -> q p e", q=128)

                # fully independent tile sets per engine pipeline (shared
                # tiles would couple the pipelines through whole-tile deps)
                for part, (eng, lo, hi) in enumerate(
                    [(nc.vector, 0, fd)] + ([(nc.gpsimd, fd, f)] if fg > 0 else [])
                ):
                    w = hi - lo
                    sfx = f"_{t}_{part}"
                    X = pool.tile([128, 9, w], dt, tag=f"X{part}", bufs=2, name="X" + sfx)
                    vb = pool.tile([128, 9, w], dt, tag=f"vb{part}", bufs=2, name="vb" + sfx)
                    nc.sync.dma_start(X[:, :, :], xsrc[:, :, lo:hi])
                    nc.sync.dma_start(vb[:, :, :], vsrc[:, :, lo:hi])
                    X4 = X.rearrange("q (a b) e -> q a b e", a=3)
                    vb4 = vb.rearrange("q (a b) e -> q a b e", a=3)

                    C = None
                    Cps = None
                    if part == 0:
                        Cps = psp.tile([128, 3, 3, w], dt, tag="Cps", name="Cps" + sfx)
                    else:
                        C = pool.tile([128, 3, 3, w], dt, tag=f"C{part}", name="C" + sfx)
                    Tb = pool.tile([128, 3, 3, w], dt, tag=f"Tb{part}", name="Tb" + sfx)
                    Wf = pool.tile([128, 3, 3, w], dt, tag=f"Wf{part}", name="Wf" + sfx)
                    sc = {
                        name: pool.tile([128, w], dt, tag=f"{name}{part}", name=f"sc_{name}{sfx}")
                        for name in scalar_names
                    }
                    sc["D"] = pool.tile([128, 3, w], dt, tag=f"D{part}", name=f"sc_D{sfx}")

                    _pipeline(nc, eng, 0, w, X4, vb4, C, Tb, Wf, sc, c_eps, c_b2, c_dl, iters, Cps=Cps)

                    nc.sync.dma_start(osrc[:, :, lo:hi], vb[:, :, :])

    nc.finalize()
    return nc


# ---------------- host side ----------------

def _to_planes(a, n_pad, fill_identity, scale=None):
    """[N,3,3] f32 -> [9, n_pad] planes (plane 3i+j = entry (i,j))."""
    n = a.shape[0]
    flat = np.empty((9, n_pad), dtype=np.float32)
    flat[:, :n] = a.reshape(n, 9).T
    if scale is not None:
        flat[:, :n] *= np.float32(scale)
    if n_pad > n:
        pad = np.zeros(9, dtype=np.float32)
        if fill_identity:
            pad[[0, 4, 8]] = 1.0
        flat[:, n:] = pad[:, None]
    return np.ascontiguousarray(flat)


_NC_CACHE = {}
LAST_RESULT = None


def _get_nc():
    key = (F, TILES, ITERS, FG)
    if key not in _NC_CACHE:
        _NC_CACHE[key] = build_nc()
    return _NC_CACHE[key]


def kernel(x, v):
    x = np.asarray(x, dtype=np.float32)
    v = np.asarray(v, dtype=np.float32)
    n = x.shape[0]
    assert n == N_TOTAL, f"expected {N_TOTAL} matrices, got {n}"

    np_tot = 128 * F * TILES
    nc = _get_nc()

    in_maps = []
    for c in range(NCORES):
        sl = slice(c * N_CORE, (c + 1) * N_CORE)
        in_maps.append(
            {
                "x": _to_planes(x[sl], np_tot, fill_identity=True),
                "v": _to_planes(v[sl], np_tot, fill_identity=False, scale=0.5),
            }
        )

    global LAST_RESULT
    res = run_bass_kernel_spmd(nc, in_maps, core_ids=list(range(NCORES)))
    LAST_RESULT = res

    outp = np.empty((n, 3, 3), dtype=np.float32)
    for c in range(NCORES):
        o = res.results[c]["out"]  # [9, np_tot]
        outp[c * N_CORE : (c + 1) * N_CORE] = (
            o[:, :N_CORE].T.reshape(N_CORE, 3, 3)
        )
    return outp


# revision 20
# speedup vs baseline: 1.3844x; 1.3844x over previous
"""Trainium2 Bass kernel: batched 3x3 polar decomposition + tangent projection.

reference semantics (per matrix n of N=2,000,000):
    u, _, vT = svd(x);  xm = u @ vT          (polar factor)
    vt = 0.5*(v - xm @ v^T @ xm)

Implementation: determinant-scaled Newton iteration for the polar factor
(gamma-form, scale-invariant):  X <- X + sign(d)|d|^(-1/3) * cof(X)
with cof() the signed cofactor matrix (X^{-T} = cof(X)/det(X)); final
iteration applies exact alpha*X + beta*cof(X) with an extra 1/sqrt(2)
folded in so the projection needs no 0.5 on the quadratic term:
    vt = 0.5 v - xmh (xmh^T v)^T,   xmh = xm/sqrt(2).

Data layout: SoA "planes" [128, 3, 3, F] per tile; the cyclic cofactor
index patterns are expressed with negative-stride access patterns
(rows (2,0) = start 2, step -2), split into 2x2 blocks per product.

Each tile's columns are split between the Vector engine (DVE) and GPSIMD,
which run the whole pipeline independently on their column ranges (fp32
tensor_tensor on DVE never takes the shared SBUF port, so both engines
stream concurrently); the Ln/Exp scalar chains run on the Scalar engine.

Sharding: batch split evenly across 8 NeuronCores, zero communication.
"""

import numpy as np

import concourse.bass as bass
import concourse.bacc as bacc
import concourse.mybir as mybir
import concourse.tile as tile
from concourse.bass_utils import run_bass_kernel_spmd

dt = mybir.dt.float32
AF = mybir.ActivationFunctionType
OP = mybir.AluOpType

NCORES = 8
N_TOTAL = 2_000_000
N_CORE = N_TOTAL // NCORES      # 250_000

# device tiling (full config)
F = 652                          # free-dim elements per partition per tile
TILES = 3
ITERS = 5                        # total Newton iterations (incl. final)
FG = 0                           # columns of each tile handled by GPSIMD

LN2 = float(np.log(2.0))
DELTA = 1e-15                    # det bump (unsticks exact-zero fp32 det)
EPS = 1e-35                      # clamp inside Ln


def _pipeline(nc, eng, lo, hi, X4, vb4, C, Tb, Wf, sc, c_eps, c_b2, c_dl, iters, Cps=None):
    """Emit the full per-tile computation for columns [lo:hi) on engine
    `eng` (nc.vector or nc.gpsimd). `sc` maps name -> [128, f] scalar tile.

    When `Cps` (a [128,3,3,hi-lo] PSUM tile) is given (DVE pipeline), the
    cofactor lives in PSUM *negated* (Cps = Tb - Ta = -cof); since gamma and
    beta are odd in det and det is computed from Cps, the two sign flips
    cancel identically. One operand of most DVE ops then comes through the
    dedicated PSUM port, leaving the shared SBUF port to GPSIMD.
    """
    fp = hi - lo
    s = lambda name: sc[name][:, lo:hi]
    X = X4[:, :, :, lo:hi]
    vb = vb4[:, :, :, lo:hi]
    Cp = Cps if Cps is not None else C[:, :, :, lo:hi]
    Tp = Tb[:, :, :, lo:hi]
    Wp = Wf[:, :, :, lo:hi]
    shp = (128, 3, 3, fp)
    psum = Cps is not None

    r12 = lambda a: a[:, 1:3, :, :]
    r20 = lambda a: a[:, 2::-2, :, :]
    r0 = lambda a: a[:, 0:1, :, :]
    r1 = lambda a: a[:, 1:2, :, :]
    c12 = lambda a: a[:, :, 1:3, :]
    c20 = lambda a: a[:, :, 2::-2, :]
    c0 = lambda a: a[:, :, 0:1, :]
    c1 = lambda a: a[:, :, 1:2, :]

    for it in range(iters):
        last = it == iters - 1

        # signed cofactor: cof = X[r1,c1]X[r2,c2] - X[r1,c2]X[r2,c1]
        # (psum path stores Cp := Tp - Ta = -cof)
        eng.tensor_mul(Cp[:, 0:2, 0:2, :], c12(r12(X)), c20(r20(X)))
        eng.tensor_mul(Cp[:, 0:2, 2:3, :], c0(r12(X)), c1(r20(X)))
        eng.tensor_mul(Cp[:, 2:3, 0:2, :], c12(r0(X)), c20(r1(X)))
        eng.tensor_mul(Cp[:, 2:3, 2:3, :], c0(r0(X)), c1(r1(X)))
        eng.tensor_mul(Tp[:, 0:2, 0:2, :], c20(r12(X)), c12(r20(X)))
        eng.tensor_mul(Tp[:, 0:2, 2:3, :], c1(r12(X)), c0(r20(X)))
        eng.tensor_mul(Tp[:, 2:3, 0:2, :], c20(r0(X)), c12(r1(X)))
        eng.tensor_mul(Tp[:, 2:3, 2:3, :], c1(r0(X)), c0(r1(X)))
        if psum:
            eng.tensor_sub(Cp, Tp, Cp)          # Cp := -cof  (in1/out PSUM)
        else:
            eng.tensor_sub(Cp, Cp, Tp)          # Cp := +cof

        # det = sum_j X[0,j]*Cp[0,j] (+ DELTA bump); sign flip is harmless
        D = sc["D"][:, :, lo:hi]
        eng.tensor_mul(D, X[:, 0, :, :], Cp[:, 0, :, :])
        eng.tensor_add(s("tq"), D[:, 0, :], D[:, 1, :])
        if eng is nc.vector:
            eng.scalar_tensor_tensor(s("ds"), s("tq"), DELTA, D[:, 2, :], OP.add, OP.add)
            eng.tensor_mul(s("d2"), s("ds"), s("ds"))
        else:
            eng.tensor_add(s("tq"), s("tq"), D[:, 2, :])
            dlb = c_dl.broadcast_to((128, fp))
            eng.tensor_add(s("ds"), s("tq"), dlb)
            eng.tensor_mul(s("d2"), s("ds"), s("ds"))
        nc.scalar.activation(s("L"), s("d2"), AF.Ln, bias=c_eps[:, :])

        if not last:
            # gamma = ds * exp(-2/3 * L)
            nc.scalar.activation(s("w"), s("L"), AF.Exp, scale=-2.0 / 3.0)
            eng.tensor_mul(s("ga"), s("ds"), s("w"))
            gb = s("ga").unsqueeze(1).unsqueeze(1).broadcast_to(shp)
            if psum:
                eng.tensor_mul(Cp, gb, Cp)      # Cp := gamma * Cp (in place)
                eng.tensor_add(X, X, Cp)
            else:
                eng.tensor_mul(Tp, Cp, gb)
                eng.tensor_add(X, X, Tp)
        else:
            # xm = alpha*X + beta*Cp (full scale)
            nc.scalar.activation(s("al"), s("L"), AF.Exp, scale=-1.0 / 6.0, bias=c_b2[:, :])
            nc.scalar.activation(s("w"), s("L"), AF.Exp, scale=-5.0 / 6.0, bias=c_b2[:, :])
            eng.tensor_mul(s("be"), s("ds"), s("w"))
            ab = s("al").unsqueeze(1).unsqueeze(1).broadcast_to(shp)
            bb = s("be").unsqueeze(1).unsqueeze(1).broadcast_to(shp)
            if psum:
                eng.tensor_mul(Cp, bb, Cp)      # beta * Cp (in place)
                eng.tensor_mul(Tp, X, ab)
                eng.tensor_add(Cp, Tp, Cp)      # xm (in PSUM)
            else:
                eng.tensor_mul(Tp, X, ab)
                eng.tensor_mul(Cp, Cp, bb)
                eng.tensor_add(Cp, Tp, Cp)
            # Cp now holds xm

    # tangent projection: vt = vh - xm (xm^T vh)^T,  vh = v/2
    for k in range(3):
        # Wf[k,j] = sum_i xm[i,k]*vh[i,j]
        ck = Cp[:, 0:3, k : k + 1, :].broadcast_to(shp)
        if psum:
            eng.tensor_mul(Tp, vb, ck)
        else:
            eng.tensor_mul(Tp, ck, vb)
        eng.tensor_add(Wp[:, k, :, :], Tp[:, 0, :, :], Tp[:, 1, :, :])
        eng.tensor_add(Wp[:, k, :, :], Wp[:, k, :, :], Tp[:, 2, :, :])
    for k in range(3):
        # P[i,j] = xm[i,k]*Wf[j,k];  out = vh - sum_k P
        cki = Cp[:, 0:3, k : k + 1, :].broadcast_to(shp)
        wkb = Wp[:, 0:3, k, :].unsqueeze(1).broadcast_to(shp)
        if psum:
            eng.tensor_mul(Tp, wkb, cki)
        else:
            eng.tensor_mul(Tp, cki, wkb)
        eng.tensor_sub(vb, vb, Tp)


def _patch_act_tables():
    """Steer the ACT table-load pass so Ln and Exp resolve to the single
    combined set (natural_log_exp_and_others); otherwise the pass picks
    separate sets and every iteration thrashes ~2.7us table loads."""
    keep = "natural_log_exp_and_others"
    orig = bacc.get_activation_tables

    def patched(arch):
        tabs = orig(arch)
        return {
            name: (funcs if name == keep else funcs - {AF.Ln, AF.Exp})
            for name, funcs in tabs.items()
        }

    bacc.get_activation_tables = patched


_patch_act_tables()


def build_nc(f=F, tiles=TILES, iters=ITERS, fg=FG):
    """Per-core Bass graph. Inputs x, v: [9, tiles*128*f] f32 planes (plane
    p = 3*i+j holds entry (i,j) of each matrix, matrix m at column m);
    output "out" same layout holding vt."""
    npt = 128 * f
    np_tot = npt * tiles
    fd = f - fg                    # DVE columns [0:fd), GPSIMD [fd:f)

    nc = bacc.Bacc()
    x = nc.declare_dram_parameter("x", [9, np_tot], dt, isOutput=False)
    v = nc.declare_dram_parameter("v", [9, np_tot], dt, isOutput=False)
    out = nc.declare_dram_parameter("out", [9, np_tot], dt, isOutput=True)

    scalar_names = ["tq", "ds", "d2", "L", "w", "ga", "al", "be"]

    with tile.TileContext(nc) as tc:
        with tc.tile_pool(name="p", bufs=1) as pool, \
             tc.tile_pool(name="ps", bufs=1, space="PSUM") as psp:
            c_eps = pool.tile([128, 1], dt, tag="c_eps")
            c_b2 = pool.tile([128, 1], dt, tag="c_b2")
            c_dl = pool.tile([128, 1], dt, tag="c_dl")
            nc.vector.memset(c_eps[:, :], EPS)
            nc.vector.memset(c_b2[:, :], -LN2)
            nc.vector.memset(c_dl[:, :], DELTA)
            for t in range(tiles):
                sl = slice(t * npt, (t + 1) * npt)
                xsrc = x[:, sl].rearrange("p (q e) -> q p e", q=128)
                vsrc = v[:, sl].rearrange("p (q e) -> q p e", q=128)
                osrc = out[:, sl].rearrange("p (q e) -> q p e", q=128)

                # fully independent tile sets per engine pipeline (shared
                # tiles would couple the pipelines through whole-tile deps)
                for part, (eng, lo, hi) in enumerate(
                    [(nc.vector, 0, fd)] + ([(nc.gpsimd, fd, f)] if fg > 0 else [])
                ):
                    w = hi - lo
                    sfx = f"_{t}_{part}"
                    X = pool.tile([128, 9, w], dt, tag=f"X{part}", bufs=2, name="X" + sfx)
                    vb = pool.tile([128, 9, w], dt, tag=f"vb{part}", bufs=2, name="vb" + sfx)
                    nc.sync.dma_start(X[:, :, :], xsrc[:, :, lo:hi])
                    nc.sync.dma_start(vb[:, :, :], vsrc[:, :, lo:hi])
                    X4 = X.rearrange("q (a b) e -> q a b e", a=3)
                    vb4 = vb.rearrange("q (a b) e -> q a b e", a=3)

                    C = None
                    Cps = None
                    if part == 0 and 9 * w * 4 <= 16384:
                        Cps = psp.tile([128, 3, 3, w], dt, tag="Cps", name="Cps" + sfx)
                    else:
                        C = pool.tile([128, 3, 3, w], dt, tag=f"C{part}", name="C" + sfx)
                    Tb = pool.tile([128, 3, 3, w], dt, tag=f"Tb{part}", name="Tb" + sfx)
                    Wf = pool.tile([128, 3, 3, w], dt, tag=f"Wf{part}", name="Wf" + sfx)
                    sc = {
                        name: pool.tile([128, w], dt, tag=f"{name}{part}", name=f"sc_{name}{sfx}")
                        for name in scalar_names
                    }
                    sc["D"] = pool.tile([128, 3, w], dt, tag=f"D{part}", name=f"sc_D{sfx}")

                    _pipeline(nc, eng, 0, w, X4, vb4, C, Tb, Wf, sc, c_eps, c_b2, c_dl, iters, Cps=Cps)

                    nc.sync.dma_start(osrc[:, :, lo:hi], vb[:, :, :])

    nc.finalize()
    return nc


# ---------------- host side ----------------

def _to_planes(a, n_pad, fill_identity, scale=None):
    """[N,3,3] f32 -> [9, n_pad] planes (plane 3i+j = entry (i,j))."""
    n = a.shape[0]
    flat = np.empty((9, n_pad), dtype=np.float32)
    flat[:, :n] = a.reshape(n, 9).T
    if scale is not None:
        flat[:, :n] *= np.float32(scale)
    if n_pad > n:
        pad = np.zeros(9, dtype=np.float32)
        if fill_identity:
            pad[[0, 4, 8]] = 1.0
        flat[:, n:] = pad[:, None]
    return np.ascontiguousarray(flat)


_NC_CACHE = {}
LAST_RESULT = None


def _get_nc():
    key = (F, TILES, ITERS, FG)
    if key not in _NC_CACHE:
        _NC_CACHE[key] = build_nc()
    return _NC_CACHE[key]


def kernel(x, v):
    x = np.asarray(x, dtype=np.float32)
    v = np.asarray(v, dtype=np.float32)
    n = x.shape[0]
    assert n == N_TOTAL, f"expected {N_TOTAL} matrices, got {n}"

    np_tot = 128 * F * TILES
    nc = _get_nc()

    in_maps = []
    for c in range(NCORES):
        sl = slice(c * N_CORE, (c + 1) * N_CORE)
        in_maps.append(
            {
                "x": _to_planes(x[sl], np_tot, fill_identity=True),
                "v": _to_planes(v[sl], np_tot, fill_identity=False, scale=0.5),
            }
        )

    global LAST_RESULT
    res = run_bass_kernel_spmd(nc, in_maps, core_ids=list(range(NCORES)))
    LAST_RESULT = res

    outp = np.empty((n, 3, 3), dtype=np.float32)
    for c in range(NCORES):
        o = res.results[c]["out"]  # [9, np_tot]
        outp[c * N_CORE : (c + 1) * N_CORE] = (
            o[:, :N_CORE].T.reshape(N_CORE, 3, 3)
        )
    return outp
